# revision 23
# baseline (speedup 1.0000x reference)
"""Trainium2 Bass kernel for nn_Encoder_36790689858290 (sparse_attention).

Strategy (8 NeuronCores), v5 — collective-free, overlap-tuned:
  Global computation (N=4, L=1024, LW=600, W=64, d=512, vd=128, S=256):
    h   = concat(x, space)                      [4096, 512]
    xn  = D @ h                                 [2400, 512]   (D = downsample)
    v'  = xn[:, :128] @ (Wo@Wv).T               (Wo folded into v)
    e.T = xnT.T-contracted against u = (Wk.T @ Wq) @ h_own.T
    A = cnt * exp(e) (cnt = host-built multiplicity matrix == the gather)
    o2[q, vd+1] accumulated DIRECTLY in [q, vd] layout (rhs = [v' | ones]),
      so Z rides along as column 128 and no PE transposes are needed.
    o2/Z ; +resid ; LN -> blk [512, 128]
    out1p = blk_own.T @ D.T[own query rows]  (host sums the 8 partials)
    out2  = xn[:, 256:512] sample rows (reused from S1; host reads even cores)

  v5 changes vs v4 (96.9us):
    - DMA ring discipline: u-inputs first on SWDGE; dp on sync ring, hp on
      scalar ring; cnt AFTER dp; dtq AFTER hp. The S1 stream gets the whole
      ~340GB/s instead of competing with late-phase tensors.
    - u computed FIRST (doubles as the HAM warm-up; no dummy warm-up MMs).
    - S1 single pass, all 8 PSUM banks (4m x 2 key-halves), kc-outer.
    - scores: o2 accumulated per-q-block [128, 129] (vd cols + Z column)
      straight from aT lhsT slices -- kills 8 PE transposes, the o2s/zs
      copies and the pzT machinery of v4.
    - LN: 4 independent chains emitted phase-major so the vector queue
      never head-of-line blocks on ACT sqrt.
    - final matmul: m-outer for m=0..2, then bank-by-bank stop pass so
      cast+DMA of bank b overlaps the matmuls of bank b+1 (v4 serialized
      ~7.5us of output DMA after the last matmul).

  All matmuls bf16 (fp32 PSUM); softmax/LN in fp32; exp stays in fp32
  range (|e| < 40 for this model). End-to-end rel err ~2e-3 vs the fp32
  reference (gate 2e-2).
"""
import os
import sys
import types

if "/opt/trn_rl_repo" not in sys.path:
    sys.path.insert(0, "/opt/trn_rl_repo")


def _ensure_ntff_hook():
    """Some container images lack antenv.axon_hooks; without it
    run_bass_kernel_spmd(trace=True) raises ImportError before it can even
    fall back. Register a shim that rebuilds the ctypes-based NTFF hook the
    boot path would have installed (degrades to no-trace if unavailable)."""
    try:
        import antenv.axon_hooks  # noqa: F401
        return
    except ImportError:
        pass
    mod = types.ModuleType("antenv.axon_hooks")

    def get_axon_ntff_profile_hook():
        try:
            from trn_agent_boot.trn_boot import _ntff_profile_via_ctypes
            return _ntff_profile_via_ctypes("/opt/axon/libaxon_pjrt.so")
        except Exception:
            return None

    mod.get_axon_ntff_profile_hook = get_axon_ntff_profile_hook
    sys.modules["antenv.axon_hooks"] = mod


_ensure_ntff_hook()

import numpy as np
import ml_dtypes

import concourse.bass as bass
import concourse.tile as tile
import concourse.mybir as mybir
from concourse.bass_utils import run_bass_kernel_spmd

BF16 = mybir.dt.bfloat16
F32 = mybir.dt.float32
NC = 8
N, L, LW, W = 4, 1024, 600, 64
D_DIM, VD, S_DIM = 512, 128, 256
GQ = N * L            # 4096 global queries
RC = (N * LW) // NC   # 300 output rows per core
SC = 2 * RC           # 600 keys per sample
QL = GQ // NC         # 512 queries per core
NKC = GQ // 128       # 32 contraction chunks of the S1 matmul
KT = 6                # key tiles of 100 partitions (6*100 = 600)
KP = 100
FB = 480              # final-matmul free-dim per PSUM bank
NFB = 5               # 5*480 = 2400 output rows

LAST_EXEC_TIME_NS = None
LAST_RESULTS = None


def _split_multi_waits(nc):
    """walrus in this image accepts at most ONE sync-wait per instruction.
    Hoist extra waits onto same-engine NOPs placed immediately before the
    instruction (engine queues execute in program order)."""
    n_split = 0
    for fn in nc.m.functions:
        for bb in fn.blocks:
            insts = list(bb.instructions)
            if not any(
                i.sync_info and i.sync_info.on_wait and len(i.sync_info.on_wait) > 1
                for i in insts
            ):
                continue
            new = []
            for inst in insts:
                si = inst.sync_info
                if si and si.on_wait and len(si.on_wait) > 1:
                    waits = list(si.on_wait)
                    for j, w in enumerate(waits[:-1]):
                        nop = mybir.InstNoOp(name=f"{inst.name}_wsplit{j}", ins=[], outs=[])
                        nop.engine = inst.engine
                        nop.sync_info = mybir.SyncInfo(on_wait=[w], on_update=[])
                        nc.register_instruction(nop)
                        new.append(nop)
                        n_split += 1
                    si.on_wait = [waits[-1]]
                    inst.sync_info = si
                new.append(inst)
            bb.instructions = new
    return n_split


def _chunk_pack(a, p=128):
    """[K, M] -> [p, K//p, M] with row g = kc*p + part."""
    k, m = a.shape
    return np.ascontiguousarray(a.reshape(k // p, p, m).transpose(1, 0, 2))


def _bf(a):
    return np.asarray(a, ml_dtypes.bfloat16)


def _build_program():
    nc = bass.Bass("TRN2", target_bir_lowering=False, debug=False, num_devices=NC)

    def din(name, shape, dt):
        return nc.dram_tensor(name, shape, dt, kind="ExternalInput").ap()

    hp = din("hp", [128, NKC, D_DIM], BF16)          # h chunks (lhsT of xn.T)
    dp = din("dp", [128, NKC, SC], BF16)             # D.T sample-600-rows chunks
    dtq = din("dtq", [128, 4, N * LW], BF16)         # D.T own-512-query rows
    htp = din("htp", [128, 4, QL], BF16)             # h.T query slice
    wkqp = din("wkqp", [128, 4, D_DIM], BF16)        # (Wk.T @ Wq).T chunks
    wvop = din("wvop", [128, VD], BF16)              # (Wo @ Wv).T
    cntp = din("cntp", [KP, KT, QL], BF16)           # cnt.T tiles (natural keys)
    resp = din("resp", [128, 4, VD], F32)            # residual (+bo folded)

    out1p = nc.dram_tensor("out1p", [VD, N * LW], BF16, kind="ExternalOutput").ap()
    out2 = nc.dram_tensor("out2", [S_DIM, SC], BF16, kind="ExternalOutput").ap()

    Exp = mybir.ActivationFunctionType.Exp
    Sqrt = mybir.ActivationFunctionType.Sqrt
    mult = mybir.AluOpType.mult
    sub = mybir.AluOpType.subtract
    add = mybir.AluOpType.add

    with tile.TileContext(nc) as tc:
        with (
            tc.tile_pool(name="big", bufs=1) as big,
            tc.tile_pool(name="tmp", bufs=2) as tmp,
            tc.tile_pool(name="pfp", bufs=5) as pfp,
            tc.tile_pool(name="psA", bufs=4, space="PSUM") as psA,
            tc.tile_pool(name="psB", bufs=4, space="PSUM") as psB,
        ):
            # ---- resident loads ---------------------------------------------
            # sync ring:   dp chunk groups, then wkq/htp (u inputs, needed
            #              only after S1), then cnt, then res.
            # scalar ring: hp chunk groups, then dtq.
            # gpsimd ring: wvo only (keeps HWDGE rings clear for S1).
            d_sb = big.tile([128, NKC, SC], BF16, tag="d_sb")
            h_sb = big.tile([128, NKC, D_DIM], BF16, tag="h_sb")
            lo = 0
            for grp in (4, 4, 8, 8, 8):
                sl = slice(lo, lo + grp)
                nc.sync.dma_start(d_sb[:, sl, :], dp[:, sl, :])
                nc.scalar.dma_start(h_sb[:, sl, :], hp[:, sl, :])
                lo += grp
            wkq_sb = big.tile([128, 4, D_DIM], BF16, tag="wkq")
            nc.sync.dma_start(wkq_sb[:], wkqp[:])
            ht_sb = big.tile([128, 4, QL], BF16, tag="ht")
            nc.sync.dma_start(ht_sb[:], htp[:])
            cnt_sb = big.tile([KP, KT, QL], BF16, tag="cnt")
            nc.sync.dma_start(cnt_sb[:], cntp[:])
            res_sb = big.tile([128, 4, VD], F32, tag="res")
            nc.sync.dma_start(res_sb[:], resp[:])
            dtq_sb = big.tile([128, 4, N * LW], BF16, tag="dtq")
            nc.scalar.dma_start(dtq_sb[:], dtq[:])
            wvo_sb = big.tile([128, VD], BF16, tag="wvo")
            nc.gpsimd.dma_start(wvo_sb[:], wvop[:])

            # constants (vector engine, no DMA)
            eps_t = big.tile([128, 1], F32, tag="eps")
            nc.vector.memset(eps_t[:], 1e-5)
            vfx_sb = big.tile([KP, KT, VD + 1], BF16, tag="vfx")
            nc.vector.memset(vfx_sb[:], 1.0)   # col 128 of each tdx stays 1.0
            warm_act = tmp.tile([1, 1], F32, tag="warm_act")
            nc.scalar.activation(warm_act[:], eps_t[0:1, :], Exp)


            # ---- S1: xnT[m] = (D[sample rows] @ h).T, [128, 600] per m ------
            # single pass, all 8 PSUM banks: 8 matmuls/chunk consume 0.28MB/us
            # against ~0.42MB/us DMA delivery, so the PE never catches the
            # stream (a 6-matmul pass1 ran at parity and stalled on every
            # group boundary).
            pxn = {}
            for m in range(4):
                pool = psA if m < 2 else psB
                for hf in range(2):
                    pxn[(m, hf)] = pool.tile(
                        [128, RC], F32, tag="A" if m < 2 else "B", name=f"pxn{m}_{hf}"
                    )
            for ms in ((0, 1, 2), (3,)):
                for kc in range(NKC):
                    for m in ms:
                        for hf in range(2):
                            nc.tensor.matmul(
                                pxn[(m, hf)][:], h_sb[:, kc, m * 128:(m + 1) * 128],
                                d_sb[:, kc, hf * RC:(hf + 1) * RC],
                                start=(kc == 0), stop=(kc == NKC - 1),
                            )
            # cast order m2,m3 first: frees the psB banks that gate u's
            # accumulators, while scores' kf0..3 ordering needs m0/m1 last
            xnT = [None] * 4
            for m in range(4):
                xnT[m] = big.tile([128, SC], BF16, tag=f"xnT{m}", name=f"xnT{m}")
                for hf in range(2):
                    cs = hf * RC
                    nc.vector.tensor_copy(xnT[m][:, cs:cs + RC], pxn[(m, hf)][:])
            # (D @ space).T sample rows -> out2, straight from the bf16 xnT
            # tiles (bf16 adds ~2e-3 on those columns; gate is 2e-2).
            for m in (2, 3):
                nc.gpsimd.dma_start(out2[(m - 2) * 128:(m - 1) * 128, :], xnT[m][:])

            # ---- v' = xn[:, :128] @ (Wo@Wv).T into vfx (ones col rides) -----
            for tdx in range(KT):
                pv = psA.tile([KP, VD], F32, tag="A", name=f"pv{tdx}")
                nc.tensor.matmul(
                    pv[:], xnT[0][:, tdx * KP:(tdx + 1) * KP], wvo_sb[:],
                    start=True, stop=True,
                )
                nc.vector.tensor_copy(vfx_sb[:, tdx, 0:VD], pv[:])

            # ---- u[kf] = ((Wk.T @ Wq) @ h_own.T)[kf-slice] ------------------
            # after S1 (PE already warm); uT casts overlap the first scores.
            uT = []
            for kf in range(4):
                pu = psB.tile([128, QL], F32, tag="B", name=f"pu{kf}")
                for a in range(4):
                    nc.tensor.matmul(
                        pu[:], wkq_sb[:, a, kf * 128:(kf + 1) * 128], ht_sb[:, a, :],
                        start=(a == 0), stop=(a == 3),
                    )
                t = big.tile([128, QL], BF16, tag=f"uT{kf}")
                nc.vector.tensor_copy(t[:], pu[:])
                uT.append(t)

            # ---- scores: e.T per key tile; A.T = cnt.T * exp(e.T);
            # o2[q, vd+1] accumulates directly in [q, vd] layout -------------
            o2acc = [
                psB.tile([128, VD + 1], F32, tag="B", name=f"o2acc{m}")
                for m in range(4)
            ]
            wu_box = []

            def dummy_mms(k):
                # keep-warm filler: PE would otherwise idle >3.4us during the
                # exp/LN chains and HAM would re-throttle the clock to 1.2GHz
                if not wu_box:  # lazy alloc so the psA ring slot frees late
                    wu_box.append(psA.tile([128, QL], F32, tag="A", name="wu"))
                for _ in range(k):
                    nc.tensor.matmul(
                        wu_box[0][:], h_sb[:, 0, 0:128], h_sb[:, 1, :],
                        start=True, stop=True,
                    )

            for tdx in range(KT):
                pe_ = psA.tile([KP, QL], F32, tag="A", name=f"pe{tdx}")
                for kf in range(4):
                    nc.tensor.matmul(
                        pe_[:], xnT[kf][:, tdx * KP:(tdx + 1) * KP], uT[kf][:],
                        start=(kf == 0), stop=(kf == 3),
                    )
                ex = tmp.tile([KP, QL], BF16, tag="ex")
                nc.scalar.activation(ex[:], pe_[:], Exp)
                aT = big.tile([KP, QL], BF16, tag=f"aT{tdx}", name=f"aT{tdx}")
                nc.vector.tensor_tensor(out=aT[:], in0=ex[:], in1=cnt_sb[:, tdx, :], op=mult)
                if tdx == KT - 1:
                    dummy_mms(3)
                for m in range(4):
                    nc.tensor.matmul(
                        o2acc[m][:], aT[:, m * 128:(m + 1) * 128], vfx_sb[:, tdx, :],
                        start=(tdx == 0), stop=(tdx == KT - 1),
                    )
            # preload the Sqrt table right after the last exp (avoids the
            # ~1.3us ACT table switch landing on the LN critical path)
            warm_sq = tmp.tile([1, 1], F32, tag="warm_sq")
            nc.scalar.activation(warm_sq[:], eps_t[0:1, :], Sqrt, bias=eps_t[0:1, :])
            dummy_mms(16)

            # ---- /Z; +resid; LayerNorm (phase-major so the 4 chains
            # pipeline across vector/scalar instead of serializing) ----------
            # ln gain/bias commute through the final D-matmul (applied on host)
            r1, mv, srt, blk = [], [], [], [None] * 4

            def ln_finish(m):
                rstd = tmp.tile([128, 1], F32, tag="rstd", name=f"rstd{m}")
                nc.vector.reciprocal(rstd[:], srt[m][:])
                blk_m = big.tile([128, VD], BF16, tag=f"blk{m}")
                nc.vector.tensor_scalar(
                    out=blk_m[:], in0=r1[m][:], scalar1=mv[m][:, 0:1], scalar2=rstd[:],
                    op0=sub, op1=mult,
                )
                blk[m] = blk_m

            for m in range(4):
                rz = tmp.tile([128, 1], F32, tag="rz", name=f"rz{m}")
                nc.vector.reciprocal(rz[:], o2acc[m][:, VD:VD + 1])
                t = big.tile([128, VD], F32, tag=f"r1_{m}")
                nc.vector.tensor_scalar(
                    out=t[:], in0=o2acc[m][:, 0:VD], scalar1=rz[:], scalar2=None,
                    op0=mult,
                )
                nc.vector.tensor_tensor(out=t[:], in0=t[:], in1=res_sb[:, m, :], op=add)
                r1.append(t)
                st = tmp.tile([128, 6], F32, tag="st", name=f"st{m}")
                nc.vector.bn_stats(st[:], t[:])
                mt = big.tile([128, 2], F32, tag=f"mv{m}")
                nc.vector.bn_aggr(mt[:], st[:])
                mv.append(mt)
                sq = tmp.tile([128, 1], F32, tag="srt", name=f"srt{m}")
                nc.scalar.activation(sq[:], mt[:, 1:2], Sqrt, bias=eps_t[:])
                srt.append(sq)
                if m >= 1:   # software-pipelined: finish m-1 while m's sqrt runs
                    ln_finish(m - 1)
            ln_finish(3)

            # ---- final: out1p = blk_own.T @ D.T[own query rows, :] -----------
            # m-outer for m=0..2 (matmuls start as each blk[m] lands), then a
            # bank-by-bank stop pass so cast+DMA of bank b overlaps bank b+1.
            pP = [
                psA.tile([128, FB], F32, tag="A", name=f"pP{b}") for b in range(4)
            ] + [psB.tile([128, FB], F32, tag="B", name="pP4")]
            for m in range(3):
                for b in range(NFB):
                    nc.tensor.matmul(
                        pP[b][:], blk[m][:], dtq_sb[:, m, b * FB:(b + 1) * FB],
                        start=(m == 0), stop=False,
                    )
            Copy = mybir.ActivationFunctionType.Copy
            for b in range(NFB):
                nc.tensor.matmul(
                    pP[b][:], blk[3][:], dtq_sb[:, 3, b * FB:(b + 1) * FB],
                    start=False, stop=True,
                )
                pf = pfp.tile([128, FB], BF16, tag="pf")
                if b % 2 == 0:   # alternate cast engines so banks drain 2-wide
                    nc.vector.tensor_copy(pf[:], pP[b][:])
                    nc.sync.dma_start(out1p[:, b * FB:(b + 1) * FB], pf[:])
                else:
                    nc.scalar.activation(pf[:], pP[b][:], Copy)
                    nc.scalar.dma_start(out1p[:, b * FB:(b + 1) * FB], pf[:])

    _split_multi_waits(nc)
    return nc


def _host_inputs(x, mask, downsample, space_pos, Wv, Wk, Wq, Wo, bo):
    x = np.asarray(x, np.float32)
    space_pos = np.asarray(space_pos, np.float32)
    downsample = np.asarray(downsample, np.float32)
    mask = np.asarray(mask)

    h = np.concatenate([x, space_pos], axis=-1).reshape(GQ, D_DIM)
    hp = _bf(_chunk_pack(h))
    hT = np.ascontiguousarray(h.T)
    DT = np.ascontiguousarray(downsample.T)

    # cnt[l, j]: multiplicity of key j in mask row l (sentinel LW dropped)
    mflat = mask.reshape(GQ, W).astype(np.int64)
    rows = np.repeat(np.arange(GQ, dtype=np.int64), W)
    cols = mflat.ravel()
    keep = cols < LW
    cnt = np.bincount(rows[keep] * LW + cols[keep], minlength=GQ * LW).reshape(
        GQ, LW
    ).astype(np.float32)

    Wkf = np.asarray(Wk, np.float32)
    Wqf = np.asarray(Wq, np.float32)
    wkq = _bf(_chunk_pack(np.ascontiguousarray((Wkf.T @ Wqf).T)))
    wvo = _bf(np.ascontiguousarray(
        (np.asarray(Wo, np.float32) @ np.asarray(Wv, np.float32)).T
    ))
    bo = np.asarray(bo, np.float32)

    dsample = [
        _bf(_chunk_pack(np.ascontiguousarray(DT[:, n * SC:(n + 1) * SC])))
        for n in range(N)
    ]
    in_maps = []
    for c in range(NC):
        n, hh = c // 2, c % 2
        htc = hT[:, c * QL:(c + 1) * QL]
        cT = cnt[n * L:(n + 1) * L].T[:, hh * QL:(hh + 1) * QL]  # [600, 512]
        cntp = _bf(np.ascontiguousarray(
            cT.reshape(KT, KP, QL).transpose(1, 0, 2)
        ))
        res = x[n, hh * QL:(hh + 1) * QL, :VD] + bo  # bo folded into residual
        in_maps.append({
            "hp": hp,
            "dp": dsample[n],
            "dtq": _bf(_chunk_pack(np.ascontiguousarray(DT[c * QL:(c + 1) * QL, :]))),
            "htp": _bf(_chunk_pack(np.ascontiguousarray(htc))),
            "wkqp": wkq, "wvop": wvo,
            "cntp": cntp,
            "resp": np.ascontiguousarray(
                res.reshape(4, 128, VD).transpose(1, 0, 2)
            ).astype(np.float32),
        })
    return in_maps


_PROGRAM = None


def _program():
    global _PROGRAM
    if _PROGRAM is None:
        _PROGRAM = _build_program()
    return _PROGRAM


def kernel(**inputs):
    global LAST_EXEC_TIME_NS, LAST_RESULTS
    in_maps = _host_inputs(
        x=inputs["x"], mask=inputs["mask"], downsample=inputs["downsample"],
        space_pos=inputs["space_pos"], Wv=inputs["Wv"], Wk=inputs["Wk"],
        Wq=inputs["Wq"], Wo=inputs["Wo"], bo=inputs["bo"],
    )
    nc = _program()
    res = run_bass_kernel_spmd(
        nc, in_maps, list(range(NC)), trace=bool(os.environ.get("KERNEL_TRACE"))
    )
    LAST_EXEC_TIME_NS = res.exec_time_ns
    LAST_RESULTS = res
    ln_g = np.asarray(inputs["ln_g"], np.float32)
    ln_b = np.asarray(inputs["ln_b"], np.float32)
    rsD = np.asarray(inputs["downsample"], np.float32).sum(axis=1)  # [2400]
    # unshard: the final matmul is contraction-sharded; sum the partials
    P = np.zeros((VD, N * LW), np.float32)
    for c in range(NC):
        P += np.asarray(res.results[c]["out1p"], np.float32)
    out = np.empty((N * LW, VD + S_DIM), np.float32)
    out[:, :VD] = P.T * ln_g[None, :] + rsD[:, None] * ln_b[None, :]
    for n in range(N):  # out2 duplicated within the pair; take even cores'
        out[n * SC:(n + 1) * SC, VD:] = np.asarray(
            res.results[2 * n]["out2"], np.float32
        ).T
    return out.reshape(N, LW, VD + S_DIM)


# revision 24
# speedup vs baseline: 1.1876x; 1.1876x over previous
"""Trainium2 Bass kernel for nn_Encoder_36790689858290 (sparse_attention).

Strategy (8 NeuronCores), v5 — collective-free, overlap-tuned:
  Global computation (N=4, L=1024, LW=600, W=64, d=512, vd=128, S=256):
    h   = concat(x, space)                      [4096, 512]
    xn  = D @ h                                 [2400, 512]   (D = downsample)
    v'  = xn[:, :128] @ (Wo@Wv).T               (Wo folded into v)
    e.T = xnT.T-contracted against u = (Wk.T @ Wq) @ h_own.T
    A = cnt * exp(e) (cnt = host-built multiplicity matrix == the gather)
    o2[q, vd+1] accumulated DIRECTLY in [q, vd] layout (rhs = [v' | ones]),
      so Z rides along as column 128 and no PE transposes are needed.
    o2/Z ; +resid ; LN -> blk [512, 128]
    out1p = blk_own.T @ D.T[own query rows]  (host sums the 8 partials)
    out2  = xn[:, 256:512] sample rows (reused from S1; host reads even cores)

  v5 changes vs v4 (96.9us):
    - DMA ring discipline: u-inputs first on SWDGE; dp on sync ring, hp on
      scalar ring; cnt AFTER dp; dtq AFTER hp. The S1 stream gets the whole
      ~340GB/s instead of competing with late-phase tensors.
    - u computed FIRST (doubles as the HAM warm-up; no dummy warm-up MMs).
    - S1 single pass, all 8 PSUM banks (4m x 2 key-halves), kc-outer.
    - scores: o2 accumulated per-q-block [128, 129] (vd cols + Z column)
      straight from aT lhsT slices -- kills 8 PE transposes, the o2s/zs
      copies and the pzT machinery of v4.
    - LN: 4 independent chains emitted phase-major so the vector queue
      never head-of-line blocks on ACT sqrt.
    - final matmul: m-outer for m=0..2, then bank-by-bank stop pass so
      cast+DMA of bank b overlaps the matmuls of bank b+1 (v4 serialized
      ~7.5us of output DMA after the last matmul).

  All matmuls bf16 (fp32 PSUM); softmax/LN in fp32; exp stays in fp32
  range (|e| < 40 for this model). End-to-end rel err ~2e-3 vs the fp32
  reference (gate 2e-2).
"""
import os
import sys
import types

if "/opt/trn_rl_repo" not in sys.path:
    sys.path.insert(0, "/opt/trn_rl_repo")


def _ensure_ntff_hook():
    """Some container images lack antenv.axon_hooks; without it
    run_bass_kernel_spmd(trace=True) raises ImportError before it can even
    fall back. Register a shim that rebuilds the ctypes-based NTFF hook the
    boot path would have installed (degrades to no-trace if unavailable)."""
    try:
        import antenv.axon_hooks  # noqa: F401
        return
    except ImportError:
        pass
    mod = types.ModuleType("antenv.axon_hooks")

    def get_axon_ntff_profile_hook():
        try:
            from trn_agent_boot.trn_boot import _ntff_profile_via_ctypes
            return _ntff_profile_via_ctypes("/opt/axon/libaxon_pjrt.so")
        except Exception:
            return None

    mod.get_axon_ntff_profile_hook = get_axon_ntff_profile_hook
    sys.modules["antenv.axon_hooks"] = mod


_ensure_ntff_hook()

import numpy as np
import ml_dtypes

import concourse.bass as bass
import concourse.tile as tile
import concourse.mybir as mybir
from concourse.bass_utils import run_bass_kernel_spmd

BF16 = mybir.dt.bfloat16
F32 = mybir.dt.float32
NC = 8
N, L, LW, W = 4, 1024, 600, 64
D_DIM, VD, S_DIM = 512, 128, 256
GQ = N * L            # 4096 global queries
RC = (N * LW) // NC   # 300 output rows per core
SC = 2 * RC           # 600 keys per sample
QL = GQ // NC         # 512 queries per core
NKC = GQ // 128       # 32 contraction chunks of the S1 matmul
KT = 6                # key tiles of 100 partitions (6*100 = 600)
KP = 100
FB = 480              # final-matmul free-dim per PSUM bank
NFB = 5               # 5*480 = 2400 output rows

LAST_EXEC_TIME_NS = None
LAST_RESULTS = None


def _split_multi_waits(nc):
    """walrus in this image accepts at most ONE sync-wait per instruction.
    Hoist extra waits onto same-engine NOPs placed immediately before the
    instruction (engine queues execute in program order)."""
    n_split = 0
    for fn in nc.m.functions:
        for bb in fn.blocks:
            insts = list(bb.instructions)
            if not any(
                i.sync_info and i.sync_info.on_wait and len(i.sync_info.on_wait) > 1
                for i in insts
            ):
                continue
            new = []
            for inst in insts:
                si = inst.sync_info
                if si and si.on_wait and len(si.on_wait) > 1:
                    waits = list(si.on_wait)
                    for j, w in enumerate(waits[:-1]):
                        nop = mybir.InstNoOp(name=f"{inst.name}_wsplit{j}", ins=[], outs=[])
                        nop.engine = inst.engine
                        nop.sync_info = mybir.SyncInfo(on_wait=[w], on_update=[])
                        nc.register_instruction(nop)
                        new.append(nop)
                        n_split += 1
                    si.on_wait = [waits[-1]]
                    inst.sync_info = si
                new.append(inst)
            bb.instructions = new
    return n_split


def _chunk_pack(a, p=128):
    """[K, M] -> [p, K//p, M] with row g = kc*p + part."""
    k, m = a.shape
    return np.ascontiguousarray(a.reshape(k // p, p, m).transpose(1, 0, 2))


def _bf(a):
    return np.asarray(a, ml_dtypes.bfloat16)


def _build_program():
    nc = bass.Bass("TRN2", target_bir_lowering=False, debug=False, num_devices=NC)

    def din(name, shape, dt):
        return nc.dram_tensor(name, shape, dt, kind="ExternalInput").ap()

    hp = din("hp", [128, NKC, D_DIM], BF16)          # h chunks (lhsT of xn.T)
    dp = din("dp", [128, NKC, SC], BF16)             # D.T sample-600-rows chunks
    dtq = din("dtq", [128, 4, N * LW], BF16)         # D.T own-512-query rows
    htp = din("htp", [128, 4, QL], BF16)             # h.T query slice
    wkqp = din("wkqp", [128, 4, D_DIM], BF16)        # (Wk.T @ Wq).T chunks
    wvop = din("wvop", [128, VD], BF16)              # (Wo @ Wv).T
    cntp = din("cntp", [KP, KT, QL], BF16)           # cnt.T tiles (natural keys)
    resp = din("resp", [128, 4, VD], F32)            # residual (+bo folded)

    out1p = nc.dram_tensor("out1p", [VD, N * LW], BF16, kind="ExternalOutput").ap()
    out2 = nc.dram_tensor("out2", [S_DIM, SC], BF16, kind="ExternalOutput").ap()

    Exp = mybir.ActivationFunctionType.Exp
    Sqrt = mybir.ActivationFunctionType.Sqrt
    mult = mybir.AluOpType.mult
    sub = mybir.AluOpType.subtract
    add = mybir.AluOpType.add

    with tile.TileContext(nc) as tc:
        with (
            tc.tile_pool(name="big", bufs=1) as big,
            tc.tile_pool(name="tmp", bufs=2) as tmp,
            tc.tile_pool(name="pfp", bufs=5) as pfp,
            tc.tile_pool(name="psA", bufs=4, space="PSUM") as psA,
            tc.tile_pool(name="psB", bufs=4, space="PSUM") as psB,
        ):
            # ---- resident loads ---------------------------------------------
            # sync ring:   dp chunk groups, then wkq/htp (u inputs, needed
            #              only after S1), then cnt, then res.
            # scalar ring: hp chunk groups, then dtq.
            # gpsimd ring: wvo only (keeps HWDGE rings clear for S1).
            d_sb = big.tile([128, NKC, SC], BF16, tag="d_sb")
            h_sb = big.tile([128, NKC, D_DIM], BF16, tag="h_sb")
            lo = 0
            for grp in (4, 4, 8, 8, 8):
                sl = slice(lo, lo + grp)
                nc.sync.dma_start(d_sb[:, sl, :], dp[:, sl, :])
                nc.scalar.dma_start(h_sb[:, sl, :], hp[:, sl, :])
                lo += grp
            wkq_sb = big.tile([128, 4, D_DIM], BF16, tag="wkq")
            nc.sync.dma_start(wkq_sb[:], wkqp[:])
            ht_sb = big.tile([128, 4, QL], BF16, tag="ht")
            nc.sync.dma_start(ht_sb[:], htp[:])
            cnt_sb = big.tile([KP, KT, QL], BF16, tag="cnt")
            nc.sync.dma_start(cnt_sb[:], cntp[:])
            res_sb = big.tile([128, 4, VD], F32, tag="res")
            nc.sync.dma_start(res_sb[:], resp[:])
            dtq_sb = big.tile([128, 4, N * LW], BF16, tag="dtq")
            nc.scalar.dma_start(dtq_sb[:], dtq[:])
            wvo_sb = big.tile([128, VD], BF16, tag="wvo")
            nc.gpsimd.dma_start(wvo_sb[:], wvop[:])

            # constants (vector engine, no DMA)
            eps_t = big.tile([128, 1], F32, tag="eps")
            nc.vector.memset(eps_t[:], 1e-5)
            vfx_sb = big.tile([KP, KT, VD + 1], BF16, tag="vfx")
            nc.vector.memset(vfx_sb[:], 1.0)   # col 128 of each tdx stays 1.0
            warm_act = tmp.tile([1, 1], F32, tag="warm_act")
            nc.scalar.activation(warm_act[:], eps_t[0:1, :], Exp)


            # ---- S1: xnT[m] = (D[sample rows] @ h).T, [128, 600] per m ------
            # single pass, all 8 PSUM banks: 8 matmuls/chunk consume 0.28MB/us
            # against ~0.42MB/us DMA delivery, so the PE never catches the
            # stream (a 6-matmul pass1 ran at parity and stalled on every
            # group boundary).
            pxn = {}
            for m in range(4):
                pool = psA if m < 2 else psB
                for hf in range(2):
                    pxn[(m, hf)] = pool.tile(
                        [128, RC], F32, tag="A" if m < 2 else "B", name=f"pxn{m}_{hf}"
                    )
            for kc in range(NKC):
                for m in range(4):
                    for hf in range(2):
                        nc.tensor.matmul(
                            pxn[(m, hf)][:], h_sb[:, kc, m * 128:(m + 1) * 128],
                            d_sb[:, kc, hf * RC:(hf + 1) * RC],
                            start=(kc == 0), stop=(kc == NKC - 1),
                        )
            # cast order m2,m3 first: frees the psB banks that gate u's
            # accumulators, while scores' kf0..3 ordering needs m0/m1 last
            xnT = [None] * 4
            for m in range(4):
                xnT[m] = big.tile([128, SC], BF16, tag=f"xnT{m}", name=f"xnT{m}")
                for hf in range(2):
                    cs = hf * RC
                    nc.vector.tensor_copy(xnT[m][:, cs:cs + RC], pxn[(m, hf)][:])
            # (D @ space).T sample rows -> out2, straight from the bf16 xnT
            # tiles (bf16 adds ~2e-3 on those columns; gate is 2e-2).
            for m in (2, 3):
                nc.gpsimd.dma_start(out2[(m - 2) * 128:(m - 1) * 128, :], xnT[m][:])

            # ---- v' = xn[:, :128] @ (Wo@Wv).T into vfx (ones col rides) -----
            for tdx in range(KT):
                pv = psA.tile([KP, VD], F32, tag="A", name=f"pv{tdx}")
                nc.tensor.matmul(
                    pv[:], xnT[0][:, tdx * KP:(tdx + 1) * KP], wvo_sb[:],
                    start=True, stop=True,
                )
                nc.vector.tensor_copy(vfx_sb[:, tdx, 0:VD], pv[:])

            # ---- u[kf] = ((Wk.T @ Wq) @ h_own.T)[kf-slice] ------------------
            # after S1 (PE already warm); uT casts overlap the first scores.
            uT = []
            for kf in range(4):
                pu = psB.tile([128, QL], F32, tag="B", name=f"pu{kf}")
                for a in range(4):
                    nc.tensor.matmul(
                        pu[:], wkq_sb[:, a, kf * 128:(kf + 1) * 128], ht_sb[:, a, :],
                        start=(a == 0), stop=(a == 3),
                    )
                t = big.tile([128, QL], BF16, tag=f"uT{kf}")
                nc.vector.tensor_copy(t[:], pu[:])
                uT.append(t)

            # ---- scores: e.T per key tile; A.T = cnt.T * exp(e.T);
            # o2[q, vd+1] accumulates directly in [q, vd] layout -------------
            o2acc = [
                psB.tile([128, VD + 1], F32, tag="B", name=f"o2acc{m}")
                for m in range(4)
            ]
            wu_box = []

            def dummy_mms(k):
                # keep-warm filler: PE would otherwise idle >3.4us during the
                # exp/LN chains and HAM would re-throttle the clock to 1.2GHz
                if not wu_box:  # lazy alloc so the psA ring slot frees late
                    wu_box.append(psA.tile([128, QL], F32, tag="A", name="wu"))
                for _ in range(k):
                    nc.tensor.matmul(
                        wu_box[0][:], h_sb[:, 0, 0:128], h_sb[:, 1, :],
                        start=True, stop=True,
                    )

            for tdx in range(KT):
                pe_ = psA.tile([KP, QL], F32, tag="A", name=f"pe{tdx}")
                for kf in range(4):
                    nc.tensor.matmul(
                        pe_[:], xnT[kf][:, tdx * KP:(tdx + 1) * KP], uT[kf][:],
                        start=(kf == 0), stop=(kf == 3),
                    )
                ex = tmp.tile([KP, QL], BF16, tag="ex")
                nc.scalar.activation(ex[:], pe_[:], Exp)
                aT = big.tile([KP, QL], BF16, tag=f"aT{tdx}", name=f"aT{tdx}")
                nc.vector.tensor_tensor(out=aT[:], in0=ex[:], in1=cnt_sb[:, tdx, :], op=mult)
                if tdx == KT - 1:
                    dummy_mms(3)
                for m in range(4):
                    nc.tensor.matmul(
                        o2acc[m][:], aT[:, m * 128:(m + 1) * 128], vfx_sb[:, tdx, :],
                        start=(tdx == 0), stop=(tdx == KT - 1),
                    )
            # preload the Sqrt table right after the last exp (avoids the
            # ~1.3us ACT table switch landing on the LN critical path)
            warm_sq = tmp.tile([1, 1], F32, tag="warm_sq")
            nc.scalar.activation(warm_sq[:], eps_t[0:1, :], Sqrt, bias=eps_t[0:1, :])
            dummy_mms(16)

            # ---- /Z; +resid; LayerNorm (phase-major so the 4 chains
            # pipeline across vector/scalar instead of serializing) ----------
            # ln gain/bias commute through the final D-matmul (applied on host)
            r1, mv, srt, blk = [], [], [], [None] * 4

            def ln_finish(m):
                rstd = tmp.tile([128, 1], F32, tag="rstd", name=f"rstd{m}")
                nc.vector.reciprocal(rstd[:], srt[m][:])
                blk_m = big.tile([128, VD], BF16, tag=f"blk{m}")
                nc.vector.tensor_scalar(
                    out=blk_m[:], in0=r1[m][:], scalar1=mv[m][:, 0:1], scalar2=rstd[:],
                    op0=sub, op1=mult,
                )
                blk[m] = blk_m

            for m in range(4):
                rz = tmp.tile([128, 1], F32, tag="rz", name=f"rz{m}")
                nc.vector.reciprocal(rz[:], o2acc[m][:, VD:VD + 1])
                t = big.tile([128, VD], F32, tag=f"r1_{m}")
                nc.vector.tensor_scalar(
                    out=t[:], in0=o2acc[m][:, 0:VD], scalar1=rz[:], scalar2=None,
                    op0=mult,
                )
                nc.vector.tensor_tensor(out=t[:], in0=t[:], in1=res_sb[:, m, :], op=add)
                r1.append(t)
                st = tmp.tile([128, 6], F32, tag="st", name=f"st{m}")
                nc.vector.bn_stats(st[:], t[:])
                mt = big.tile([128, 2], F32, tag=f"mv{m}")
                nc.vector.bn_aggr(mt[:], st[:])
                mv.append(mt)
                sq = tmp.tile([128, 1], F32, tag="srt", name=f"srt{m}")
                nc.scalar.activation(sq[:], mt[:, 1:2], Sqrt, bias=eps_t[:])
                srt.append(sq)
                if m >= 1:   # software-pipelined: finish m-1 while m's sqrt runs
                    ln_finish(m - 1)
            ln_finish(3)

            # ---- final: out1p = blk_own.T @ D.T[own query rows, :] -----------
            # m-outer for m=0..2 (matmuls start as each blk[m] lands), then a
            # bank-by-bank stop pass so cast+DMA of bank b overlaps bank b+1.
            pP = [
                psA.tile([128, FB], F32, tag="A", name=f"pP{b}") for b in range(4)
            ] + [psB.tile([128, FB], F32, tag="B", name="pP4")]
            for m in range(3):
                for b in range(NFB):
                    nc.tensor.matmul(
                        pP[b][:], blk[m][:], dtq_sb[:, m, b * FB:(b + 1) * FB],
                        start=(m == 0), stop=False,
                    )
            Copy = mybir.ActivationFunctionType.Copy
            for b in range(NFB):
                nc.tensor.matmul(
                    pP[b][:], blk[3][:], dtq_sb[:, 3, b * FB:(b + 1) * FB],
                    start=False, stop=True,
                )
                pf = pfp.tile([128, FB], BF16, tag="pf")
                if b % 2 == 0:   # alternate cast engines so banks drain 2-wide
                    nc.vector.tensor_copy(pf[:], pP[b][:])
                    nc.sync.dma_start(out1p[:, b * FB:(b + 1) * FB], pf[:])
                else:
                    nc.scalar.activation(pf[:], pP[b][:], Copy)
                    nc.scalar.dma_start(out1p[:, b * FB:(b + 1) * FB], pf[:])

    _split_multi_waits(nc)
    return nc


def _host_inputs(x, mask, downsample, space_pos, Wv, Wk, Wq, Wo, bo):
    x = np.asarray(x, np.float32)
    space_pos = np.asarray(space_pos, np.float32)
    downsample = np.asarray(downsample, np.float32)
    mask = np.asarray(mask)

    h = np.concatenate([x, space_pos], axis=-1).reshape(GQ, D_DIM)
    hp = _bf(_chunk_pack(h))
    hT = np.ascontiguousarray(h.T)
    DT = np.ascontiguousarray(downsample.T)

    # cnt[l, j]: multiplicity of key j in mask row l (sentinel LW dropped)
    mflat = mask.reshape(GQ, W).astype(np.int64)
    rows = np.repeat(np.arange(GQ, dtype=np.int64), W)
    cols = mflat.ravel()
    keep = cols < LW
    cnt = np.bincount(rows[keep] * LW + cols[keep], minlength=GQ * LW).reshape(
        GQ, LW
    ).astype(np.float32)

    Wkf = np.asarray(Wk, np.float32)
    Wqf = np.asarray(Wq, np.float32)
    wkq = _bf(_chunk_pack(np.ascontiguousarray((Wkf.T @ Wqf).T)))
    wvo = _bf(np.ascontiguousarray(
        (np.asarray(Wo, np.float32) @ np.asarray(Wv, np.float32)).T
    ))
    bo = np.asarray(bo, np.float32)

    dsample = [
        _bf(_chunk_pack(np.ascontiguousarray(DT[:, n * SC:(n + 1) * SC])))
        for n in range(N)
    ]
    in_maps = []
    for c in range(NC):
        n, hh = c // 2, c % 2
        htc = hT[:, c * QL:(c + 1) * QL]
        cT = cnt[n * L:(n + 1) * L].T[:, hh * QL:(hh + 1) * QL]  # [600, 512]
        cntp = _bf(np.ascontiguousarray(
            cT.reshape(KT, KP, QL).transpose(1, 0, 2)
        ))
        res = x[n, hh * QL:(hh + 1) * QL, :VD] + bo  # bo folded into residual
        in_maps.append({
            "hp": hp,
            "dp": dsample[n],
            "dtq": _bf(_chunk_pack(np.ascontiguousarray(DT[c * QL:(c + 1) * QL, :]))),
            "htp": _bf(_chunk_pack(np.ascontiguousarray(htc))),
            "wkqp": wkq, "wvop": wvo,
            "cntp": cntp,
            "resp": np.ascontiguousarray(
                res.reshape(4, 128, VD).transpose(1, 0, 2)
            ).astype(np.float32),
        })
    return in_maps


_PROGRAM = None


def _program():
    global _PROGRAM
    if _PROGRAM is None:
        _PROGRAM = _build_program()
    return _PROGRAM


def kernel(**inputs):
    global LAST_EXEC_TIME_NS, LAST_RESULTS
    in_maps = _host_inputs(
        x=inputs["x"], mask=inputs["mask"], downsample=inputs["downsample"],
        space_pos=inputs["space_pos"], Wv=inputs["Wv"], Wk=inputs["Wk"],
        Wq=inputs["Wq"], Wo=inputs["Wo"], bo=inputs["bo"],
    )
    nc = _program()
    res = run_bass_kernel_spmd(
        nc, in_maps, list(range(NC)), trace=bool(os.environ.get("KERNEL_TRACE"))
    )
    LAST_EXEC_TIME_NS = res.exec_time_ns
    LAST_RESULTS = res
    ln_g = np.asarray(inputs["ln_g"], np.float32)
    ln_b = np.asarray(inputs["ln_b"], np.float32)
    rsD = np.asarray(inputs["downsample"], np.float32).sum(axis=1)  # [2400]
    # unshard: the final matmul is contraction-sharded; sum the partials
    P = np.zeros((VD, N * LW), np.float32)
    for c in range(NC):
        P += np.asarray(res.results[c]["out1p"], np.float32)
    out = np.empty((N * LW, VD + S_DIM), np.float32)
    out[:, :VD] = P.T * ln_g[None, :] + rsD[:, None] * ln_b[None, :]
    for n in range(N):  # out2 duplicated within the pair; take even cores'
        out[n * SC:(n + 1) * SC, VD:] = np.asarray(
            res.results[2 * n]["out2"], np.float32
        ).T
    return out.reshape(N, LW, VD + S_DIM)
